# revision 1
# baseline (speedup 1.0000x reference)
"""Trainium2 Bass kernel for nn_MultiHeadAttention_38233798869424.

Reference computation (B=4, S=2048, IN=OUT=1024, H=16, D=64):
    q = x @ Wq; k = x @ Wk; v = x @ Wv            # [B, S, 1024]
    q,k,v -> reshape(B, H, S, D)   (PLAIN view, no transpose!)
    attn per (b, h): softmax(q k^T / 8) v          # [B, H, S, D]
    ctx -> reshape(B, S, 1024); out = ctx @ Wp + bp

Key structural insight: the plain reshape means "head" h of batch b attends
only within rows [h*128, (h+1)*128) of x[b] — i.e. the problem decomposes
into B*H = 64 fully independent blocks, each a self-attention over 2048
"positions" (p = 16*r + cgroup) of dim 64 built from a [128, 1024] slice of
x. We shard the 64 blocks 8-per-core (pure SPMD, no collectives) and
compute each block's attention in a permuted position order p~ = g*128 + r
(g = column-group) which is softmax-invariant and makes every matmul
operand a contiguous/strided AP with no transposes:

  per block j (128 rows of x):
    Q~T,K~T [64(d), 2048(p~)] = (Wq|Wk col-packed)^T-chunks @ x^T-chunks
    V       [128(r), 1024]    = x-chunk @ Wv (g-th 64-col slice == V~ chunk)
    S~T[kp, qp] tiles [128, 2048] = K~T-slice.T @ Q~T      (f32r, K=64)
    expS = exp(S~T/8) (no max-subtraction needed: |s|/8 <~ 4)   [fp16]
    ctx~T_aug [65, 2048] += V_aug-chunk.T @ expS-chunk  (ones col -> sums)
    ctx~T *= 1/sums  (PE outer-product broadcast + DVE)
    out_block [128, 1024] = sum_g ctx~T[:, g-slice].T @ Wp[g-rows] + bp
"""

from contextlib import ExitStack

import numpy as np

import concourse.bass as bass

B, S, IN_DIM, OUT_DIM, H = 4, 2048, 1024, 1024, 16
D = OUT_DIM // H  # 64
N_CORES = 8
BLOCKS_PER_CORE = (B * H) // N_CORES  # 8
WAVE = 4  # blocks per wave (SBUF residency for Q/K)
N_WAVES = BLOCKS_PER_CORE // WAVE  # 2
KC = IN_DIM // 128  # 8 contraction chunks
G = 16  # column groups per block


def _build_nc():
    import concourse.tile as tile
    from concourse import bacc, mybir

    F32 = mybir.dt.float32
    F32R = mybir.dt.float32r
    F16 = mybir.dt.float16
    EXP = mybir.ActivationFunctionType.Exp

    nc = bacc.Bacc("TRN2")
    xt_d = nc.dram_tensor("xt", [128, KC * 1024], F16, kind="ExternalInput")
    wqk_d = nc.dram_tensor("wqk", [128, KC * 2048], F16, kind="ExternalInput")
    wv_d = nc.dram_tensor("wv", [128, KC * 1024], F16, kind="ExternalInput")
    wp_d = nc.dram_tensor("wp", [128, KC * 1024], F16, kind="ExternalInput")
    bp_d = nc.dram_tensor("bp", [1, 1024], F32R, kind="ExternalInput")
    ones_d = nc.dram_tensor("ones", [1, 128], F32R, kind="ExternalInput")
    out_d = nc.dram_tensor("out", [1024, 1024], F32, kind="ExternalOutput")

    with tile.TileContext(nc) as tc, ExitStack() as ctx:
        const = ctx.enter_context(tc.tile_pool(name="const", bufs=1))
        wavep = ctx.enter_context(tc.tile_pool(name="wavep", bufs=1))
        work = ctx.enter_context(tc.tile_pool(name="work", bufs=1))
        ps = ctx.enter_context(tc.tile_pool(name="ps", bufs=1, space="PSUM"))

        # chunked loads so the first matmuls can start before the whole
        # weight set has landed
        wqk_sb = const.tile([128, KC * 2048], F16)
        xt_sb = const.tile([128, KC * 1024], F16)
        wv_sb = const.tile([128, KC * 1024], F16)
        for kc in range(KC):
            nc.sync.dma_start(
                xt_sb[:, kc * 1024 : (kc + 1) * 1024],
                xt_d[:, kc * 1024 : (kc + 1) * 1024],
            )
            nc.sync.dma_start(
                wqk_sb[:, kc * 2048 : (kc + 1) * 2048],
                wqk_d[:, kc * 2048 : (kc + 1) * 2048],
            )
        for kc in range(KC):
            nc.sync.dma_start(
                wv_sb[:, kc * 1024 : (kc + 1) * 1024],
                wv_d[:, kc * 1024 : (kc + 1) * 1024],
            )
        # Wp in 128-row contraction chunks: chunk i holds rows [i*128,
        # (i+1)*128) — even g-group on partitions 0:64, odd on 64:128,
        # which is exactly what the row-packed projection pair needs.
        wp_sb = const.tile([128, KC * 1024], F16)
        nc.sync.dma_start(wp_sb, wp_d[:, :])
        bp_sb = const.tile([1, 1024], F32R)
        nc.sync.dma_start(bp_sb, bp_d[:, :])
        ones_sb = const.tile([1, 128], F32R)
        nc.sync.dma_start(ones_sb, ones_d[:, :])

        # graduated wave sizes: a tiny first wave so startup only serializes
        # one block's projections; later phase-1 work hides under the
        # previous wave's (ACT-bound) attention sweep
        wave_sizes = [4, 4]
        wave_starts = [sum(wave_sizes[:k]) for k in range(len(wave_sizes))]
        for j0, wn in zip(wave_starts, wave_sizes):
            # ---- phase 1: QKV projections for the wn blocks of this wave --
            # qk_sb free layout: j4*2048 + g*128 + r; parts 0:64 = Q~T(d),
            # parts 64:128 = K~T(d).
            qk_sb = wavep.tile([128, WAVE * 2048], F32R, tag="qk")
            # v_sb free layout: j4*1040 + g*65 + d, with an all-ones column
            # at d=64 of each g (feeds the softmax-denominator row).
            v_sb = wavep.tile([128, WAVE * 1040], F16, tag="v")
            # ones in the d=64 column of every g-group (overwritten at 0:64
            # by the V copies below) -> softmax denominator row of ctx~T.
            nc.vector.memset(v_sb, 1.0)

            for g in range(G):
                qk_ps = ps.tile([128, wn * 128], F32, tag="s", bufs=2)
                for kc in range(KC):
                    nc.tensor.matmul(
                        qk_ps,
                        lhsT=wqk_sb[:, kc * 2048 + g * 128 : kc * 2048 + g * 128 + 128],
                        rhs=xt_sb[:, kc * 1024 + j0 * 128 : kc * 1024 + (j0 + wn) * 128],
                        start=(kc == 0),
                        stop=(kc == KC - 1),
                    )
                # scatter [128, (j4)(r)] -> qk_sb[:, j4*2048 + g*128 + r]
                out_view = qk_sb.rearrange("p (j f) -> p j f", j=WAVE)[
                    :, 0:wn, g * 128 : g * 128 + 128
                ]
                in_view = qk_ps.rearrange("p (j f) -> p j f", j=wn)
                nc.vector.tensor_copy(out_view, in_view)

            for j4 in range(wn):
                for ns in range(2):
                    v_ps = ps.tile([128, 512], F32, tag="s", bufs=2)
                    for kc in range(KC):
                        nc.tensor.matmul(
                            v_ps,
                            lhsT=xt_sb[
                                :,
                                kc * 1024 + (j0 + j4) * 128 : kc * 1024
                                + (j0 + j4) * 128
                                + 128,
                            ],
                            rhs=wv_sb[:, kc * 1024 + ns * 512 : kc * 1024 + ns * 512 + 512],
                            start=(kc == 0),
                            stop=(kc == KC - 1),
                        )
                    # [128, (8g)(64d)] -> v_sb[:, j4*1040 + (ns*8+g)*65 + d]
                    o = v_sb.rearrange("p (a e) -> p a e", e=65)[
                        :, j4 * 16 + ns * 8 : j4 * 16 + ns * 8 + 8, 0:64
                    ]
                    nc.vector.tensor_copy(o, v_ps.rearrange("p (a e) -> p a e", e=64))

            # ---- phase 2: attention per block ----
            for j4 in range(wn):
                j = j0 + j4
                # k2: K~T chunk pairs packed on complementary partition
                # halves — even kpos-chunks on 0:64, odd on 64:128 — so two
                # K=64 scores matmuls run concurrently via row tiling.
                k2 = work.tile([128, 1024], F32R, tag="k2", bufs=2)
                ksrc = qk_sb[64:128, j4 * 2048 : j4 * 2048 + 2048].rearrange(
                    "p (i two r) -> p i two r", two=2, r=128
                )
                k2lo = k2[0:64, :].rearrange("p (i r) -> p i r", r=128)
                k2hi = k2[64:128, :].rearrange("p (i r) -> p i r", r=128)
                nc.sync.dma_start(k2lo, ksrc[:, :, 0, :])
                nc.sync.dma_start(k2hi, ksrc[:, :, 1, :])
                # Q~T duplicated onto partitions 64:128 (rhs of the B-side)
                qq = work.tile([128, 2048], F32R, tag="qq", bufs=1)
                nc.sync.dma_start(
                    qq[64:128, :], qk_sb[0:64, j4 * 2048 : j4 * 2048 + 2048]
                )

                ctxT_sb = work.tile([128, 2048], F16, tag="ctxT", bufs=2)
                for h in range(2):
                    # per-half ctx accumulator: [65, 1024] = 2 PSUM banks so
                    # halves/blocks can overlap (tag "c" also holds psA)
                    ctx_ps = ps.tile([65, 1024], F32, tag="c", bufs=2)
                    for i in range(G // 2):
                        sA = ps.tile([128, 1024], F32, tag="s", bufs=2)
                        sB = ps.tile([128, 1024], F32, tag="s", bufs=2)
                        for ns in range(2):
                            q_off = j4 * 2048 + h * 1024 + ns * 512
                            nc.tensor.matmul(
                                sA[:, ns * 512 : ns * 512 + 512],
                                lhsT=k2[0:64, i * 128 : i * 128 + 128],
                                rhs=qk_sb[0:64, q_off : q_off + 512],
                                start=True,
                                stop=True,
                                tile_position=(0, 0),
                            )
                            q_off2 = h * 1024 + ns * 512
                            nc.tensor.matmul(
                                sB[:, ns * 512 : ns * 512 + 512],
                                lhsT=k2[64:128, i * 128 : i * 128 + 128],
                                rhs=qq[64:128, q_off2 : q_off2 + 512],
                                start=True,
                                stop=True,
                                tile_position=(64, 0),
                            )
                        esA = work.tile([128, 1024], F16, tag="es", bufs=6)
                        nc.scalar.activation(esA, sA, EXP, scale=0.125)
                        esB = work.tile([128, 1024], F16, tag="es", bufs=6)
                        nc.scalar.activation(esB, sB, EXP, scale=0.125)
                        for gk, es in ((2 * i, esA), (2 * i + 1, esB)):
                            for ns in range(2):
                                nc.tensor.matmul(
                                    ctx_ps[:, ns * 512 : ns * 512 + 512],
                                    lhsT=v_sb[
                                        :,
                                        j4 * 1040 + gk * 65 : j4 * 1040 + gk * 65 + 65,
                                    ],
                                    rhs=es[:, ns * 512 : ns * 512 + 512],
                                    start=(gk == 0),
                                    stop=(gk == G - 1),
                                )
                    # normalize: 1/sums (row 64), gpsimd partition-broadcast,
                    # one DVE multiply into fp16 ctx~T
                    inv_sb = work.tile([1, 1024], F32, tag="inv", bufs=2)
                    nc.vector.reciprocal(inv_sb, ctx_ps[64:65, :])
                    invb = work.tile([64, 1024], F32, tag="invb", bufs=2)
                    nc.gpsimd.partition_broadcast(invb, inv_sb)
                    nc.vector.tensor_mul(
                        ctxT_sb[0:64, h * 1024 : h * 1024 + 1024],
                        ctx_ps[0:64, :],
                        invb,
                    )
                    # duplicate ctx~T onto partitions 64:128 for the
                    # row-packed projection pair
                    nc.sync.dma_start(
                        ctxT_sb[64:128, h * 1024 : h * 1024 + 1024],
                        ctxT_sb[0:64, h * 1024 : h * 1024 + 1024],
                    )

                # final projection: row-packed pairs — even contraction
                # chunks accumulate in psA (+bias), odd in psB, added on DVE
                psA = ps.tile([128, 1024], F32, tag="c", bufs=2)
                psB = ps.tile([128, 1024], F32, tag="s", bufs=2)
                for i in range(KC):
                    for ns in range(2):
                        nc.tensor.matmul(
                            psA[:, ns * 512 : ns * 512 + 512],
                            lhsT=ctxT_sb[0:64, (2 * i) * 128 : (2 * i) * 128 + 128],
                            rhs=wp_sb[0:64, i * 1024 + ns * 512 : i * 1024 + ns * 512 + 512],
                            start=(i == 0),
                            stop=False,
                            tile_position=(0, 0),
                        )
                        nc.tensor.matmul(
                            psB[:, ns * 512 : ns * 512 + 512],
                            lhsT=ctxT_sb[64:128, (2 * i + 1) * 128 : (2 * i + 1) * 128 + 128],
                            rhs=wp_sb[64:128, i * 1024 + ns * 512 : i * 1024 + ns * 512 + 512],
                            start=(i == 0),
                            stop=(i == KC - 1),
                            tile_position=(64, 0),
                        )
                for ns in range(2):
                    nc.tensor.matmul(
                        psA[:, ns * 512 : ns * 512 + 512],
                        lhsT=ones_sb[:, 0:128],
                        rhs=bp_sb[:, ns * 512 : ns * 512 + 512],
                        start=False,
                        stop=True,
                    )
                obf = work.tile([128, 1024], F32, tag="obf", bufs=2)
                nc.vector.tensor_copy(obf, psB)
                out_sb = work.tile([128, 1024], F32, tag="outsb", bufs=2)
                nc.vector.tensor_add(out_sb, psA, obf)
                nc.sync.dma_start(out_d[j * 128 : j * 128 + 128, :], out_sb)

    nc.compile()
    return nc


_compiled = {}


def kernel(x, Wq, Wk, Wv, Wp, bp):
    from concourse.bass_utils import run_bass_kernel_spmd

    x = np.asarray(x, dtype=np.float32)
    Wq = np.asarray(Wq, dtype=np.float32)
    Wk = np.asarray(Wk, dtype=np.float32)
    Wv = np.asarray(Wv, dtype=np.float32)
    Wp = np.asarray(Wp, dtype=np.float32)
    bp = np.asarray(bp, dtype=np.float32)

    f16 = np.float16

    # weights, shared by all cores
    wqk = np.empty((IN_DIM, G, 128), np.float32)
    wqk[:, :, :64] = Wq.reshape(IN_DIM, G, 64)
    wqk[:, :, 64:] = Wk.reshape(IN_DIM, G, 64)
    wqk_sb = (
        wqk.reshape(KC, 128, 2048).transpose(1, 0, 2).reshape(128, KC * 2048)
    ).astype(f16)
    wv_sb = (
        Wv.reshape(KC, 128, 1024).transpose(1, 0, 2).reshape(128, KC * 1024)
    ).astype(f16)
    wp_sb = (
        Wp.reshape(KC, 128, 1024).transpose(1, 0, 2).reshape(128, KC * 1024)
    ).astype(f16)
    bp_sb = bp.reshape(1, 1024).astype(np.float32)

    x_flat = x.reshape(B * S, IN_DIM)
    in_maps = []
    for c in range(N_CORES):
        slab = x_flat[c * 1024 : (c + 1) * 1024]  # [1024 rows, 1024 k]
        xt = np.ascontiguousarray(slab.T)  # [k, jr]
        xt_sb = (
            xt.reshape(KC, 128, 1024).transpose(1, 0, 2).reshape(128, KC * 1024)
        ).astype(f16)
        in_maps.append(
            {
                "xt": xt_sb,
                "wqk": wqk_sb,
                "wv": wv_sb,
                "wp": wp_sb,
                "bp": bp_sb,
                "ones": np.ones((1, 128), np.float32),
            }
        )

    if "nc" not in _compiled:
        _compiled["nc"] = _build_nc()
    nc = _compiled["nc"]

    res = run_bass_kernel_spmd(nc, in_maps, list(range(N_CORES)))

    out = np.empty((B * S, OUT_DIM), np.float32)
    for c in range(N_CORES):
        out[c * 1024 : (c + 1) * 1024] = res.results[c]["out"]
    return out.reshape(B, S, OUT_DIM)



# revision 8
# speedup vs baseline: 1.1995x; 1.1995x over previous
"""Trainium2 Bass kernel for nn_MultiHeadAttention_38233798869424.

Reference computation (B=4, S=2048, IN=OUT=1024, H=16, D=64):
    q = x @ Wq; k = x @ Wk; v = x @ Wv            # [B, S, 1024]
    q,k,v -> reshape(B, H, S, D)   (PLAIN view, no transpose!)
    attn per (b, h): softmax(q k^T / 8) v          # [B, H, S, D]
    ctx -> reshape(B, S, 1024); out = ctx @ Wp + bp

Structural insight (same as the earlier kernel): the plain reshape means
"head" h of batch b attends only within rows [h*128, (h+1)*128) of x[b] --
the problem decomposes into B*H = 64 fully independent 128-row blocks, each
a self-attention over 2048 positions of dim 64.  8 blocks per core, pure
SPMD, no collectives.  Positions are processed in the softmax-invariant
permuted order p~ = g*128 + r (g = column group 0..15, r = row 0..127).

This version restructures the per-block compute so every matmul hits the
cost-model optimum (cost = out_free_size per 128-contraction chunk) and the
whole thing runs as a block-level software pipeline that keeps the ACT
engine (exp: the true roofline, ~34us/block) and the PE busy together:

  per block j (128 rows of x):
    K~T [64(d), 2048(p~)], Q~T staged, V [128(r), 16x65(g,d+ones)]
        <- 6 PSUM chunks, drained by DVE; Q~T moved to partitions 0:64
           by a small SBUF->SBUF DMA (q2).
    per q-half (1024 q), per k-tile i (128 kpos):
      S~T tile [128 kpos, 1024 q] = K~T_i.T @ Q~ (two N=512 matmuls, K=64)
      es = exp(S~T/8) -> fp16                        (ACT, the bottleneck)
      ctx[q-tile 128, 65] += es_tile.T @ V_i[128,65]  (K=128, N=65; ones
          column accumulates the softmax denominator)
    normalize: ctx *= 1/sums  (DVE per-partition scalar broadcast)
    ctx^T via PE transpose-matmuls, stacked so partitions 0:64 hold even-g
        d's and 64:128 odd-g d's  ->  K=128 output projection
    out[128, 1024] = sum_i ctxT2_i.T @ Wp_i + bp  (8 single K=128 chunks)
"""

from collections import deque
from contextlib import ExitStack

import numpy as np

import concourse.bass as bass

B, S, IN_DIM, OUT_DIM, H = 4, 2048, 1024, 1024, 16
D = OUT_DIM // H  # 64
N_CORES = 8
NBLK = (B * H) // N_CORES  # 8 blocks per core
KC = IN_DIM // 128  # 8 contraction chunks
G = 16  # column groups per block (k-tiles)


def _build_nc():
    import concourse.tile as tile
    from concourse import bacc, mybir

    F32 = mybir.dt.float32
    F32R = mybir.dt.float32r
    F16 = mybir.dt.float16
    EXP = mybir.ActivationFunctionType.Exp

    nc = bacc.Bacc("TRN2")
    xt_d = nc.dram_tensor("xt", [128, KC * 1024], F16, kind="ExternalInput")
    wqk_d = nc.dram_tensor("wqk", [128, KC * 2048], F16, kind="ExternalInput")
    wv_d = nc.dram_tensor("wv", [128, KC * 1024], F16, kind="ExternalInput")
    wp_d = nc.dram_tensor("wp", [128, KC * 1024], F16, kind="ExternalInput")
    bp_d = nc.dram_tensor("bp", [1, 1024], F32R, kind="ExternalInput")
    ones_d = nc.dram_tensor("ones", [1, 128], F32R, kind="ExternalInput")
    eye_d = nc.dram_tensor("eye", [128, 128], F16, kind="ExternalInput")
    out_d = nc.dram_tensor("out", [1024, 1024], F32, kind="ExternalOutput")

    with tile.TileContext(nc) as tc, ExitStack() as ctx:
        const = ctx.enter_context(tc.tile_pool(name="const", bufs=1))
        work = ctx.enter_context(tc.tile_pool(name="work", bufs=1))
        ps = ctx.enter_context(tc.tile_pool(name="ps", bufs=1, space="PSUM"))

        # chunked weight loads so the first projections can start early
        wqk_sb = const.tile([128, KC * 2048], F16)
        xt_sb = const.tile([128, KC * 1024], F16)
        wv_sb = const.tile([128, KC * 1024], F16)
        for kc in range(KC):
            nc.sync.dma_start(
                xt_sb[:, kc * 1024 : (kc + 1) * 1024],
                xt_d[:, kc * 1024 : (kc + 1) * 1024],
            )
            nc.sync.dma_start(
                wqk_sb[:, kc * 2048 : (kc + 1) * 2048],
                wqk_d[:, kc * 2048 : (kc + 1) * 2048],
            )
        for kc in range(KC):
            nc.sync.dma_start(
                wv_sb[:, kc * 1024 : (kc + 1) * 1024],
                wv_d[:, kc * 1024 : (kc + 1) * 1024],
            )
        wp_sb = const.tile([128, KC * 1024], F16)
        nc.sync.dma_start(wp_sb, wp_d[:, :])
        bp_sb = const.tile([1, 1024], F32R)
        nc.sync.dma_start(bp_sb, bp_d[:, :])
        ones_sb = const.tile([1, 128], F32R)
        nc.sync.dma_start(ones_sb, ones_d[:, :])
        eye_sb = const.tile([128, 128], F16)
        nc.sync.dma_start(eye_sb, eye_d[:, :])

        # ---- per-block rotating SBUF state -------------------------------
        # kq: partitions 0:64 = K~T [d, p~], 64:128 = Q~T staging (same
        #     columns); q2 = Q~T moved to partitions 0:64 via DMA.
        def kq_tile():
            return work.tile([128, 2048], F16, tag="kq", bufs=2, name="kq")

        def q2_tile():
            return work.tile([64, 2048], F16, tag="q2", bufs=2, name="q2")

        def v_tile():
            return work.tile([128, G * 65], F16, tag="v", bufs=2, name="vsb")

        blk = {}  # j -> (kq, q2, v_sb)

        # ---- projection emission (block j), in 6 PE chunks ----------------
        # chunk c in 0..3: QK for g-groups [4c, 4c+4); c in 4..5: V halves.
        def emit_proj_chunk(j, c):
            kq, q2, v_sb = blk[j]
            if c < 4:
                qk_ps = ps.tile([128, 512], F32, tag="mis", name="mis")
                for gl in range(4):
                    g = c * 4 + gl
                    for kc in range(KC):
                        nc.tensor.matmul(
                            qk_ps[:, gl * 128 : (gl + 1) * 128],
                            lhsT=wqk_sb[
                                :, kc * 2048 + g * 128 : kc * 2048 + (g + 1) * 128
                            ],
                            rhs=xt_sb[:, kc * 1024 + j * 128 : kc * 1024 + (j + 1) * 128],
                            start=(kc == 0),
                            stop=(kc == KC - 1),
                        )
                nc.vector.tensor_copy(
                    kq[0:64, c * 512 : (c + 1) * 512], qk_ps[0:64, :]
                )
                nc.vector.tensor_copy(
                    kq[64:128, c * 512 : (c + 1) * 512], qk_ps[64:128, :]
                )
                nc.sync.dma_start(
                    q2[:, c * 512 : (c + 1) * 512], kq[64:128, c * 512 : (c + 1) * 512]
                )
            else:
                ns = c - 4
                if ns == 0:
                    # ones column at d=64 of every g (softmax denominator);
                    # overwritten at 0:64 by the V copies below
                    nc.vector.memset(v_sb, 1.0)
                v_ps = ps.tile([128, 512], F32, tag="mis", name="mis")
                for kc in range(KC):
                    nc.tensor.matmul(
                        v_ps,
                        lhsT=xt_sb[:, kc * 1024 + j * 128 : kc * 1024 + (j + 1) * 128],
                        rhs=wv_sb[:, kc * 1024 + ns * 512 : kc * 1024 + ns * 512 + 512],
                        start=(kc == 0),
                        stop=(kc == KC - 1),
                    )
                o = v_sb.rearrange("p (a e) -> p a e", e=65)[
                    :, ns * 8 : (ns + 1) * 8, 0:64
                ]
                nc.vector.tensor_copy(o, v_ps.rearrange("p (a e) -> p a e", e=64))

        # proj chunks of block j+1 are interleaved into attention of block j
        # at these (qh, i) steps:
        projsched = {
            (0, 2): 0,
            (0, 5): 1,
            (0, 8): 2,
            (0, 11): 3,
            (0, 14): 4,
            (1, 2): 5,
        }

        # ---- attention for one block (software pipelined on PE) -----------
        def emit_attention(j, next_j):
            kq, q2, v_sb = blk[j]
            ctxT2 = work.tile([128, 1024], F16, tag="ctxT2", bufs=2, name="ctxT2")
            pv_lag = deque()  # (qh, i, es_tile, ctxE, ctxO)
            ctx_cur = {}
            es0_cur = {}

            # PSUM start=True marks the WHOLE 2KB bank pending-zero, so of
            # the 4 interleaved accumulation slots per ctx bank only the
            # last-started one (tl 6/7) keeps its i=0 term; the others get
            # it re-added by emit_readd() at the end of the q-half.
            def emit_pv(qh, i, es_t, ctxE, ctxO):
                for tl in range(8):
                    ctx_t = ctxE if tl % 2 == 0 else ctxO
                    sl = (tl // 2) * 65
                    nc.tensor.matmul(
                        ctx_t[:, sl : sl + 65],
                        lhsT=es_t[:, tl * 128 : (tl + 1) * 128],
                        rhs=v_sb[:, i * 65 : i * 65 + 65],
                        start=(i == 0),
                        stop=(i == G - 1 and tl >= 6),
                    )

            def emit_readd(qh, ctxE, ctxO):
                es0 = es0_cur[qh]
                for tl in range(6):
                    ctx_t = ctxE if tl % 2 == 0 else ctxO
                    sl = (tl // 2) * 65
                    nc.tensor.matmul(
                        ctx_t[:, sl : sl + 65],
                        lhsT=es0[:, tl * 128 : (tl + 1) * 128],
                        rhs=v_sb[:, 0:65],
                        start=False,
                        stop=True,
                    )

            def emit_norm_tr(qh, ctxE, ctxO):
                ctxn = work.tile([128, 512], F16, tag="ctxn", bufs=2, name="ctxn")
                for tl in range(8):
                    ctx_t = ctxE if tl % 2 == 0 else ctxO
                    sl = (tl // 2) * 65
                    inv = work.tile([128, 1], F32, tag="inv", bufs=4, name="inv")
                    nc.vector.reciprocal(inv, ctx_t[:, sl + 64 : sl + 65])
                    nc.vector.tensor_scalar_mul(
                        ctxn[:, tl * 64 : (tl + 1) * 64],
                        ctx_t[:, sl : sl + 64],
                        inv,
                    )
                psT2 = ps.tile([128, 512], F16, tag="pt", name="pt")
                for tl in range(8):
                    t = qh * 8 + tl  # global g of this q-tile
                    dst = (
                        psT2[0:64, (tl // 2) * 128 : (tl // 2) * 128 + 128]
                        if t % 2 == 0
                        else psT2[64:128, (tl // 2) * 128 : (tl // 2) * 128 + 128]
                    )
                    nc.tensor.transpose(dst, ctxn[:, tl * 64 : (tl + 1) * 64], eye_sb)
                nc.vector.tensor_copy(ctxT2[:, qh * 512 : (qh + 1) * 512], psT2)

            for qh in range(2):
                ctxE = ps.tile([128, 512], F32, tag="ctxE", name="ctxE")
                ctxO = ps.tile([128, 512], F32, tag="ctxO", name="ctxO")
                ctx_cur[qh] = (ctxE, ctxO)
                for i in range(G):
                    s_t = ps.tile([128, 1024], F32, tag="s", bufs=2, name="st")
                    for half in range(2):
                        nc.tensor.matmul(
                            s_t[:, half * 512 : (half + 1) * 512],
                            lhsT=kq[0:64, i * 128 : (i + 1) * 128],
                            rhs=q2[:, qh * 1024 + half * 512 : qh * 1024 + half * 512 + 512],
                            start=True,
                            stop=True,
                        )
                    if i == 0:
                        # pinned: re-read at the end of the q-half by
                        # emit_readd (the rotating "es" tag would be gone)
                        es_t = work.tile(
                            [128, 1024], F16, tag="es0", bufs=2, name="es0"
                        )
                    else:
                        es_t = work.tile(
                            [128, 1024], F16, tag="es", bufs=3, name="es"
                        )
                    nc.scalar.activation(es_t, s_t, EXP, scale=0.125)
                    if i == 0:
                        es0_cur[qh] = es_t
                    pv_lag.append((qh, i, es_t, ctxE, ctxO))

                    if (qh, i) in projsched and next_j is not None:
                        emit_proj_chunk(next_j, projsched[(qh, i)])
                    # PE runs PV two steps behind scores so the exp (ACT) of
                    # the lagged tile has drained by the time PE needs it
                    if len(pv_lag) > 2:
                        emit_pv(*pv_lag.popleft())
                    # once qh0's last PV has been emitted (during qh1 step 1),
                    # normalize+transpose qh0 so its ctx banks free up for qh1
                    if qh == 1 and i == 1:
                        emit_readd(0, *ctx_cur[0])
                        emit_norm_tr(0, *ctx_cur[0])
                # keep the 2-deep lag across the qh0->qh1 boundary
            # flush remaining PV, then normalize+transpose qh1
            while pv_lag:
                emit_pv(*pv_lag.popleft())
            emit_readd(1, *ctx_cur[1])
            emit_norm_tr(1, *ctx_cur[1])

            # output projection: 8 single K=128 chunks per 512-col half
            out_sb = work.tile([128, 1024], F32, tag="osb", bufs=2, name="osb")
            for hlf in range(2):
                psO = ps.tile([128, 512], F32, tag="mis", name="mis")
                for i in range(KC):
                    nc.tensor.matmul(
                        psO,
                        lhsT=ctxT2[:, i * 128 : (i + 1) * 128],
                        rhs=wp_sb[:, i * 1024 + hlf * 512 : i * 1024 + hlf * 512 + 512],
                        start=(i == 0),
                        stop=False,
                    )
                nc.tensor.matmul(
                    psO,
                    lhsT=ones_sb[:, 0:128],
                    rhs=bp_sb[:, hlf * 512 : hlf * 512 + 512],
                    start=False,
                    stop=True,
                )
                nc.vector.tensor_copy(out_sb[:, hlf * 512 : (hlf + 1) * 512], psO)
            nc.sync.dma_start(out_d[j * 128 : (j + 1) * 128, :], out_sb)

        # ---- pipeline ----------------------------------------------------
        blk[0] = (kq_tile(), q2_tile(), v_tile())
        for c in range(6):
            emit_proj_chunk(0, c)
        for j in range(NBLK):
            next_j = j + 1 if j + 1 < NBLK else None
            if next_j is not None:
                blk[next_j] = (kq_tile(), q2_tile(), v_tile())
            emit_attention(j, next_j)
            del blk[j]

    nc.compile()
    return nc


_compiled = {}


def kernel(x, Wq, Wk, Wv, Wp, bp):
    from concourse.bass_utils import run_bass_kernel_spmd

    x = np.asarray(x, dtype=np.float32)
    Wq = np.asarray(Wq, dtype=np.float32)
    Wk = np.asarray(Wk, dtype=np.float32)
    Wv = np.asarray(Wv, dtype=np.float32)
    Wp = np.asarray(Wp, dtype=np.float32)
    bp = np.asarray(bp, dtype=np.float32)

    f16 = np.float16

    # weights, shared by all cores.  wqk group g: cols 0:64 = Wk (scores
    # lhsT wants K~T on partitions 0:64), 64:128 = Wq.
    wqk = np.empty((IN_DIM, G, 128), np.float32)
    wqk[:, :, :64] = Wk.reshape(IN_DIM, G, 64)
    wqk[:, :, 64:] = Wq.reshape(IN_DIM, G, 64)
    wqk_sb = (
        wqk.reshape(KC, 128, 2048).transpose(1, 0, 2).reshape(128, KC * 2048)
    ).astype(f16)
    wv_sb = (
        Wv.reshape(KC, 128, 1024).transpose(1, 0, 2).reshape(128, KC * 1024)
    ).astype(f16)
    wp_sb = (
        Wp.reshape(KC, 128, 1024).transpose(1, 0, 2).reshape(128, KC * 1024)
    ).astype(f16)
    bp_sb = bp.reshape(1, 1024).astype(np.float32)
    eye = np.eye(128, dtype=f16)

    x_flat = x.reshape(B * S, IN_DIM)
    in_maps = []
    for c in range(N_CORES):
        slab = x_flat[c * 1024 : (c + 1) * 1024]  # [1024 rows, 1024 k]
        xt = np.ascontiguousarray(slab.T)  # [k, jr]
        xt_sb = (
            xt.reshape(KC, 128, 1024).transpose(1, 0, 2).reshape(128, KC * 1024)
        ).astype(f16)
        in_maps.append(
            {
                "xt": xt_sb,
                "wqk": wqk_sb,
                "wv": wv_sb,
                "wp": wp_sb,
                "bp": bp_sb,
                "ones": np.ones((1, 128), np.float32),
                "eye": eye,
            }
        )

    if "nc" not in _compiled:
        _compiled["nc"] = _build_nc()
    nc = _compiled["nc"]

    res = run_bass_kernel_spmd(nc, in_maps, list(range(N_CORES)))

    out = np.empty((B * S, OUT_DIM), np.float32)
    for c in range(N_CORES):
        out[c * 1024 : (c + 1) * 1024] = res.results[c]["out"]
    return out.reshape(B, S, OUT_DIM)


# revision 11
# speedup vs baseline: 1.2721x; 1.0605x over previous
"""Trainium2 Bass kernel for nn_MultiHeadAttention_38233798869424.

Reference computation (B=4, S=2048, IN=OUT=1024, H=16, D=64):
    q = x @ Wq; k = x @ Wk; v = x @ Wv            # [B, S, 1024]
    q,k,v -> reshape(B, H, S, D)   (PLAIN view, no transpose!)
    attn per (b, h): softmax(q k^T / 8) v          # [B, H, S, D]
    ctx -> reshape(B, S, 1024); out = ctx @ Wp + bp

The plain reshape means "head" h of batch b attends only within rows
[h*128, (h+1)*128) of x[b]: the problem decomposes into B*H = 64 fully
independent 128-row blocks, each a self-attention over 2048 positions of
dim 64.  8 blocks per core, pure SPMD, no collectives.  Positions are
processed in the softmax-invariant permuted order p~ = g*128 + r
(g = column group 0..15, r = row 0..127).

Engine budget per core (cost model): ACT exp = 267us (hard floor: exp only
runs on ACT at 1 elem/lane/cycle), PE matmuls = 286us.  The emission is a
flat software pipeline paced by the 32 score-tiles per block: each "step"
emits the score matmuls for one [128 kpos, 1024 q] tile, the PV matmuls of
the step two back (so the ACT exp has drained), and at most ~0.5us of
other PE work (projection micro-chunks for the NEXT block, output
projection halves of the PREVIOUS block, transposes), so the ACT engine is
never starved and the PE never sits on a lumpy dependency.

Per block j:
  K~T [64(d), 2048(p~)], Q~T staged in kq[64:128] -> q2 via one DMA,
      V [128(r), 16x65(g,d+ones)]: 16 per-g QK micro-chunks (8 matmuls,
      one [128,128] DVE drain each) + 4 V quarter-chunks.
  per q-half, per k-tile i: S~T tile = K~T_i.T @ Q~ (2 N=512 matmuls,
      K=64), es = exp(S~T/8) (ACT), ctx[q-tile, 65] += es_tl.T @ V_i
      (K=128, N=65; the ones column accumulates the softmax denominator).
      PSUM start=True poisons the whole 2KB bank, so of the 4 interleaved
      ctx slots per bank only the last-started keeps its i=0 term; the
      others get it re-added at the end of the half (emit_readd).
  normalize ctx by 1/sums (DVE per-partition scalar), PE-transpose into
      psT2 stacked [128 = even-g d | odd-g d, 4x128 r], DVE-stack into
      ctxT2 -> 8 single K=128 output-projection chunks + matmul bias,
      split into two half-contractions so transposes/stacks can hide
      between them; the second output half spills into the next window.
"""

from collections import deque
from contextlib import ExitStack

import numpy as np

import concourse.bass as bass

B, S, IN_DIM, OUT_DIM, H = 4, 2048, 1024, 1024, 16
D = OUT_DIM // H  # 64
N_CORES = 8
NBLK = (B * H) // N_CORES  # 8 blocks per core
KC = IN_DIM // 128  # 8 contraction chunks
G = 16  # column groups per block (k-tiles)


def _build_nc():
    import concourse.tile as tile
    from concourse import bacc, mybir

    F32 = mybir.dt.float32
    F32R = mybir.dt.float32r
    F16 = mybir.dt.float16
    EXP = mybir.ActivationFunctionType.Exp

    nc = bacc.Bacc("TRN2")
    # xt2: block-major  [128, j*1024 + kc*128 + r]
    xt_d = nc.dram_tensor("xt", [128, NBLK * 1024], F16, kind="ExternalInput")
    # wqk2: g-major  [128, g*1024 + kc*128 + dd]  (dd: 0:64 = Wk, 64:128 = Wq)
    wqk_d = nc.dram_tensor("wqk", [128, G * 1024], F16, kind="ExternalInput")
    wv_d = nc.dram_tensor("wv", [128, KC * 1024], F16, kind="ExternalInput")
    wp_d = nc.dram_tensor("wp", [128, KC * 1024], F16, kind="ExternalInput")
    bp_d = nc.dram_tensor("bp", [1, 1024], F32R, kind="ExternalInput")
    ones_d = nc.dram_tensor("ones", [1, 128], F32R, kind="ExternalInput")
    eye_d = nc.dram_tensor("eye", [128, 128], F16, kind="ExternalInput")
    out_d = nc.dram_tensor("out", [1024, 1024], F32, kind="ExternalOutput")

    with tile.TileContext(nc) as tc, ExitStack() as ctx:
        const = ctx.enter_context(tc.tile_pool(name="const", bufs=1))
        work = ctx.enter_context(tc.tile_pool(name="work", bufs=1))
        ps = ctx.enter_context(tc.tile_pool(name="ps", bufs=1, space="PSUM"))

        xt_sb = const.tile([128, NBLK * 1024], F16)
        wqk_sb = const.tile([128, G * 1024], F16)
        wv_sb = const.tile([128, KC * 1024], F16)
        wp_sb = const.tile([128, KC * 1024], F16)
        # DMA order = first-consumption order: x/weights for block 0's
        # projection stream first, everything else behind it.
        nc.sync.dma_start(xt_sb[:, 0:1024], xt_d[:, 0:1024])
        for g in range(G):
            nc.sync.dma_start(
                wqk_sb[:, g * 1024 : (g + 1) * 1024],
                wqk_d[:, g * 1024 : (g + 1) * 1024],
            )
        for kc in range(KC):
            nc.sync.dma_start(
                wv_sb[:, kc * 1024 : (kc + 1) * 1024],
                wv_d[:, kc * 1024 : (kc + 1) * 1024],
            )
        eye_sb = const.tile([128, 128], F16)
        nc.sync.dma_start(eye_sb, eye_d[:, :])
        for j in range(1, NBLK):
            nc.sync.dma_start(
                xt_sb[:, j * 1024 : (j + 1) * 1024], xt_d[:, j * 1024 : (j + 1) * 1024]
            )
        nc.sync.dma_start(wp_sb, wp_d[:, :])
        bp_sb = const.tile([1, 1024], F32R)
        nc.sync.dma_start(bp_sb, bp_d[:, :])
        ones_sb = const.tile([1, 128], F32R)
        nc.sync.dma_start(ones_sb, ones_d[:, :])

        blk = {}  # j -> (kq, q2, v_sb)

        def alloc_blk(j):
            blk[j] = (
                work.tile([128, 2048], F16, tag="kq", bufs=2, name="kq"),
                work.tile([64, 2048], F16, tag="q2", bufs=2, name="q2"),
                work.tile([128, G * 65], F16, tag="v", bufs=2, name="vsb"),
            )

        # ---- projection micro-items (20 per block) ------------------------
        # ('qk', g): QK for group g -> kq[:, g*128:+128]; ('v', q): quarter.
        def emit_proj_item(j, it):
            kq, q2, v_sb = blk[j]
            kind, idx = it
            if kind == "qk":
                g = idx
                qk_ps = ps.tile([128, 512], F32, tag="mis", name="mis")
                sl = (g % 4) * 128
                for kc in range(KC):
                    nc.tensor.matmul(
                        qk_ps[:, sl : sl + 128],
                        lhsT=wqk_sb[:, g * 1024 + kc * 128 : g * 1024 + (kc + 1) * 128],
                        rhs=xt_sb[:, j * 1024 + kc * 128 : j * 1024 + (kc + 1) * 128],
                        start=(kc == 0),
                        stop=(kc == KC - 1),
                    )
                nc.vector.tensor_copy(
                    kq[:, g * 128 : (g + 1) * 128], qk_ps[:, sl : sl + 128]
                )
                if g == G - 1:
                    nc.sync.dma_start(q2, kq[64:128, :])
            else:
                qtr = idx
                if qtr == 0:
                    nc.vector.memset(v_sb, 1.0)
                v_ps = ps.tile([128, 512], F32, tag="mis", name="mis")
                sl = (qtr % 2) * 256
                for kc in range(KC):
                    nc.tensor.matmul(
                        v_ps[:, sl : sl + 256],
                        lhsT=xt_sb[:, j * 1024 + kc * 128 : j * 1024 + (kc + 1) * 128],
                        rhs=wv_sb[:, kc * 1024 + qtr * 256 : kc * 1024 + (qtr + 1) * 256],
                        start=(kc == 0),
                        stop=(kc == KC - 1),
                    )
                o = v_sb.rearrange("p (a e) -> p a e", e=65)[
                    :, qtr * 4 : (qtr + 1) * 4, 0:64
                ]
                nc.vector.tensor_copy(
                    o, v_ps[:, sl : sl + 256].rearrange("p (a e) -> p a e", e=64)
                )

        def proj_items():
            return [("qk", g) for g in range(G)] + [("v", q) for q in range(4)]

        # ---- per-block attention pieces -----------------------------------
        def emit_scores(j, qh, i):
            kq, q2, _ = blk[j]
            s_t = ps.tile([128, 1024], F32, tag="s", bufs=2, name="st")
            for half in range(2):
                nc.tensor.matmul(
                    s_t[:, half * 512 : (half + 1) * 512],
                    lhsT=kq[0:64, i * 128 : (i + 1) * 128],
                    rhs=q2[:, qh * 1024 + half * 512 : qh * 1024 + half * 512 + 512],
                    start=True,
                    stop=True,
                )
            if i == 0:
                es_t = work.tile([128, 1024], F16, tag="es0", bufs=2, name="es0")
            else:
                es_t = work.tile([128, 1024], F16, tag="es", bufs=3, name="es")
            nc.scalar.activation(es_t, s_t, EXP, scale=0.125)
            return es_t

        def emit_pv(j, qh, i, es_t, ctxE, ctxO):
            v_sb = blk[j][2]
            for tl in range(8):
                ctx_t = ctxE if tl % 2 == 0 else ctxO
                sl = (tl // 2) * 65
                nc.tensor.matmul(
                    ctx_t[:, sl : sl + 65],
                    lhsT=es_t[:, tl * 128 : (tl + 1) * 128],
                    rhs=v_sb[:, i * 65 : i * 65 + 65],
                    start=(i == 0),
                    stop=(i == G - 1 and tl >= 6),
                )

        def emit_readd(j, es0, ctxE, ctxO):
            v_sb = blk[j][2]
            for tl in range(6):
                ctx_t = ctxE if tl % 2 == 0 else ctxO
                sl = (tl // 2) * 65
                nc.tensor.matmul(
                    ctx_t[:, sl : sl + 65],
                    lhsT=es0[:, tl * 128 : (tl + 1) * 128],
                    rhs=v_sb[:, 0:65],
                    start=False,
                    stop=True,
                )

        def emit_norm(ctxE, ctxO):
            ctxn = work.tile([128, 512], F16, tag="ctxn", bufs=2, name="ctxn")
            for tl in range(8):
                ctx_t = ctxE if tl % 2 == 0 else ctxO
                sl = (tl // 2) * 65
                inv = work.tile([128, 1], F32, tag="inv", bufs=4, name="inv")
                nc.vector.reciprocal(inv, ctx_t[:, sl + 64 : sl + 65])
                nc.vector.tensor_scalar_mul(
                    ctxn[:, tl * 64 : (tl + 1) * 64], ctx_t[:, sl : sl + 64], inv
                )
            return ctxn

        def emit_tr(qh, ctxn):
            psT2 = ps.tile([128, 512], F16, tag="pt", name="pt")
            for tl in range(8):
                t = qh * 8 + tl
                dst = (
                    psT2[0:64, (tl // 2) * 128 : (tl // 2) * 128 + 128]
                    if t % 2 == 0
                    else psT2[64:128, (tl // 2) * 128 : (tl // 2) * 128 + 128]
                )
                nc.tensor.transpose(dst, ctxn[:, tl * 64 : (tl + 1) * 64], eye_sb)
            return psT2

        def emit_stack(qh, psT2, ctxT2):
            nc.vector.tensor_copy(ctxT2[:, qh * 512 : (qh + 1) * 512], psT2)

        def emit_outproj_half(hlf, part, ctxT2, psO):
            # part 0: contraction chunks 0..3 (start); part 1: 4..7 + bias
            for i in range(part * 4, part * 4 + 4):
                nc.tensor.matmul(
                    psO,
                    lhsT=ctxT2[:, i * 128 : (i + 1) * 128],
                    rhs=wp_sb[:, i * 1024 + hlf * 512 : i * 1024 + hlf * 512 + 512],
                    start=(i == 0),
                    stop=False,
                )
            if part == 1:
                nc.tensor.matmul(
                    psO,
                    lhsT=ones_sb[:, 0:128],
                    rhs=bp_sb[:, hlf * 512 : hlf * 512 + 512],
                    start=False,
                    stop=True,
                )

        # ---- flat pipeline ------------------------------------------------
        # per window j: 32 score steps; PV lags 2; proj items of block j+1
        # spread one per step; block-(j-1) tail consumed at steps 0/1;
        # block-end chain pre-emits the first two score steps of block j+1.
        STEPS = [(qh, i) for qh in range(2) for i in range(G)]

        alloc_blk(0)
        for it in proj_items():
            emit_proj_item(0, it)

        tail = None  # (j, ctxT2, out_sb) pending second output half
        pre_scored = {}  # (j, qh, i) -> es tile, for steps emitted early

        for j in range(NBLK):
            kq, q2, v_sb = blk[j]
            ctxT2 = work.tile([128, 1024], F16, tag="ctxT2", bufs=2, name="ctxT2")
            out_sb = work.tile([128, 1024], F32, tag="osb", bufs=2, name="osb")
            next_j = j + 1 if j + 1 < NBLK else None
            if next_j is not None:
                alloc_blk(next_j)
            pitems = deque(proj_items() if next_j is not None else [])
            # proj item steps: qh0 i=2..15, qh1 i=2..7
            pslots = {(0, i) for i in range(2, 16)} | {(1, i) for i in range(2, 8)}

            pv_lag = deque()
            ctx_cur = {}
            es0_cur = {}
            ctxn_cur = {}

            for qh, i in STEPS:
                if i == 0:
                    ctx_cur[qh] = (
                        ps.tile([128, 512], F32, tag="ctxE", name="ctxE"),
                        ps.tile([128, 512], F32, tag="ctxO", name="ctxO"),
                    )
                if (j, qh, i) in pre_scored:
                    es_t = pre_scored.pop((j, qh, i))
                else:
                    es_t = emit_scores(j, qh, i)
                if i == 0:
                    es0_cur[qh] = es_t
                pv_lag.append((qh, i, es_t))

                if tail is not None and qh == 0 and i in (0, 1):
                    # previous block's second output half + store
                    tj, tctxT2, tout, tpsO = tail
                    if i == 0:
                        tpsO = ps.tile([128, 512], F32, tag="mis", name="mis")
                        tail = (tj, tctxT2, tout, tpsO)
                    emit_outproj_half(1, i, tctxT2, tpsO)
                    if i == 1:
                        nc.vector.tensor_copy(tout[:, 512:1024], tpsO)
                        nc.sync.dma_start(out_d[tj * 128 : (tj + 1) * 128, :], tout)
                        tail = None
                if pitems and (qh, i) in pslots:
                    emit_proj_item(next_j, pitems.popleft())
                if len(pv_lag) > 2:
                    pqh, pi, pes = pv_lag.popleft()
                    emit_pv(j, pqh, pi, pes, *ctx_cur[pqh])
                if qh == 1 and i == 1:
                    # qh0 fully accumulated (PV(0,15) just emitted above)
                    emit_readd(j, es0_cur[0], *ctx_cur[0])
                    ctxn_cur[0] = emit_norm(*ctx_cur[0])
                if qh == 1 and i == 3:
                    psT2 = emit_tr(0, ctxn_cur[0])
                    emit_stack(0, psT2, ctxT2)

            # ---- block-end chain ----
            while pv_lag:
                pqh, pi, pes = pv_lag.popleft()
                emit_pv(j, pqh, pi, pes, *ctx_cur[pqh])
            emit_readd(j, es0_cur[1], *ctx_cur[1])
            # first output half, chunks 0..3 (qh0 data): fills the norm gap
            psO = ps.tile([128, 512], F32, tag="mis", name="mis")
            emit_outproj_half(0, 0, ctxT2, psO)
            ctxn1 = emit_norm(*ctx_cur[1])
            psT2 = emit_tr(1, ctxn1)
            if next_j is not None:
                pre_scored[(next_j, 0, 0)] = emit_scores(next_j, 0, 0)
            emit_stack(1, psT2, ctxT2)
            emit_outproj_half(0, 1, ctxT2, psO)
            nc.vector.tensor_copy(out_sb[:, 0:512], psO)
            if next_j is not None:
                pre_scored[(next_j, 0, 1)] = emit_scores(next_j, 0, 1)
                tail = (j, ctxT2, out_sb, None)
            else:
                psO = ps.tile([128, 512], F32, tag="mis", name="mis")
                emit_outproj_half(1, 0, ctxT2, psO)
                emit_outproj_half(1, 1, ctxT2, psO)
                nc.vector.tensor_copy(out_sb[:, 512:1024], psO)
                nc.sync.dma_start(out_d[j * 128 : (j + 1) * 128, :], out_sb)
            del blk[j]

    nc.compile()
    return nc


_compiled = {}


def kernel(x, Wq, Wk, Wv, Wp, bp):
    from concourse.bass_utils import run_bass_kernel_spmd

    x = np.asarray(x, dtype=np.float32)
    Wq = np.asarray(Wq, dtype=np.float32)
    Wk = np.asarray(Wk, dtype=np.float32)
    Wv = np.asarray(Wv, dtype=np.float32)
    Wp = np.asarray(Wp, dtype=np.float32)
    bp = np.asarray(bp, dtype=np.float32)

    f16 = np.float16

    # wqk2 g-major: [c, g, dd] -> [128 (c%128... c = kc*128+p), g*1024 + kc*128 + dd]
    wqk = np.empty((IN_DIM, G, 128), np.float32)
    wqk[:, :, :64] = Wk.reshape(IN_DIM, G, 64)
    wqk[:, :, 64:] = Wq.reshape(IN_DIM, G, 64)
    # [kc, p, g, dd] -> p on partitions, cols g*1024 + kc*128 + dd
    wqk_sb = (
        wqk.reshape(KC, 128, G, 128).transpose(1, 2, 0, 3).reshape(128, G * 1024)
    ).astype(f16)
    wv_sb = (
        Wv.reshape(KC, 128, 1024).transpose(1, 0, 2).reshape(128, KC * 1024)
    ).astype(f16)
    wp_sb = (
        Wp.reshape(KC, 128, 1024).transpose(1, 0, 2).reshape(128, KC * 1024)
    ).astype(f16)
    bp_sb = bp.reshape(1, 1024).astype(np.float32)
    eye = np.eye(128, dtype=f16)

    x_flat = x.reshape(B * S, IN_DIM)
    in_maps = []
    for c in range(N_CORES):
        slab = x_flat[c * 1024 : (c + 1) * 1024]  # [1024 rows, 1024 c]
        xt = np.ascontiguousarray(slab.T)  # [c, row]
        # block-major: [kc, p, j, r] -> [128, j*1024 + kc*128 + r]
        xt_sb = (
            xt.reshape(KC, 128, NBLK, 128).transpose(1, 2, 0, 3).reshape(128, NBLK * 1024)
        ).astype(f16)
        in_maps.append(
            {
                "xt": xt_sb,
                "wqk": wqk_sb,
                "wv": wv_sb,
                "wp": wp_sb,
                "bp": bp_sb,
                "ones": np.ones((1, 128), np.float32),
                "eye": eye,
            }
        )

    if "nc" not in _compiled:
        _compiled["nc"] = _build_nc()
    nc = _compiled["nc"]

    res = run_bass_kernel_spmd(nc, in_maps, list(range(N_CORES)))

    out = np.empty((B * S, OUT_DIM), np.float32)
    for c in range(N_CORES):
        out[c * 1024 : (c + 1) * 1024] = res.results[c]["out"]
    return out.reshape(B, S, OUT_DIM)


# revision 12
# speedup vs baseline: 1.2880x; 1.0125x over previous
"""Trainium2 Bass kernel for nn_MultiHeadAttention_38233798869424.

Reference computation (B=4, S=2048, IN=OUT=1024, H=16, D=64):
    q = x @ Wq; k = x @ Wk; v = x @ Wv            # [B, S, 1024]
    q,k,v -> reshape(B, H, S, D)   (PLAIN view, no transpose!)
    attn per (b, h): softmax(q k^T / 8) v          # [B, H, S, D]
    ctx -> reshape(B, S, 1024); out = ctx @ Wp + bp

The plain reshape means "head" h of batch b attends only within rows
[h*128, (h+1)*128) of x[b]: the problem decomposes into B*H = 64 fully
independent 128-row blocks, each a self-attention over 2048 positions of
dim 64.  8 blocks per core, pure SPMD, no collectives.  Positions are
processed in the softmax-invariant permuted order p~ = g*128 + r
(g = column group 0..15, r = row 0..127).

Engine budget per core (cost model): ACT exp = 267us (hard floor: exp only
runs on ACT at 1 elem/lane/cycle), PE matmuls = 286us.  The emission is a
flat software pipeline paced by the 32 score-tiles per block: each "step"
emits the score matmuls for one [128 kpos, 1024 q] tile, the PV matmuls of
the step two back (so the ACT exp has drained), and at most ~0.5us of
other PE work (projection micro-chunks for the NEXT block, output
projection halves of the PREVIOUS block, transposes), so the ACT engine is
never starved and the PE never sits on a lumpy dependency.

Per block j:
  K~T [64(d), 2048(p~)], Q~T staged in kq[64:128] -> q2 via one DMA,
      V [128(r), 16x65(g,d+ones)]: 16 per-g QK micro-chunks (8 matmuls,
      one [128,128] DVE drain each) + 4 V quarter-chunks.
  per q-half, per k-tile i: S~T tile = K~T_i.T @ Q~ (2 N=512 matmuls,
      K=64), es = exp(S~T/8) (ACT), ctx[q-tile, 65] += es_tl.T @ V_i
      (K=128, N=65; the ones column accumulates the softmax denominator).
      PSUM start=True poisons the whole 2KB bank, so of the 4 interleaved
      ctx slots per bank only the last-started keeps its i=0 term; the
      others get it re-added at the end of the half (emit_readd).
  normalize ctx by 1/sums (DVE per-partition scalar), PE-transpose into
      psT2 stacked [128 = even-g d | odd-g d, 4x128 r], DVE-stack into
      ctxT2 -> 8 single K=128 output-projection chunks + matmul bias,
      split into two half-contractions so transposes/stacks can hide
      between them; the second output half spills into the next window.
"""

from collections import deque
from contextlib import ExitStack

import numpy as np

import concourse.bass as bass

B, S, IN_DIM, OUT_DIM, H = 4, 2048, 1024, 1024, 16
D = OUT_DIM // H  # 64
N_CORES = 8
NBLK = (B * H) // N_CORES  # 8 blocks per core
KC = IN_DIM // 128  # 8 contraction chunks
G = 16  # column groups per block (k-tiles)


def _build_nc():
    import concourse.tile as tile
    from concourse import bacc, mybir

    F32 = mybir.dt.float32
    F32R = mybir.dt.float32r
    F16 = mybir.dt.float16
    EXP = mybir.ActivationFunctionType.Exp

    nc = bacc.Bacc("TRN2")
    # xt2: block-major  [128, j*1024 + kc*128 + r]
    xt_d = nc.dram_tensor("xt", [128, NBLK * 1024], F16, kind="ExternalInput")
    # wqk2: g-major  [128, g*1024 + kc*128 + dd]  (dd: 0:64 = Wk, 64:128 = Wq)
    wqk_d = nc.dram_tensor("wqk", [128, G * 1024], F16, kind="ExternalInput")
    wv_d = nc.dram_tensor("wv", [128, 4 * 2048], F16, kind="ExternalInput")
    wp_d = nc.dram_tensor("wp", [128, KC * 1024], F16, kind="ExternalInput")
    bp_d = nc.dram_tensor("bp", [1, 1024], F32R, kind="ExternalInput")
    ones_d = nc.dram_tensor("ones", [1, 128], F32R, kind="ExternalInput")
    eye_d = nc.dram_tensor("eye", [128, 128], F16, kind="ExternalInput")
    out_d = nc.dram_tensor("out", [1024, 1024], F32, kind="ExternalOutput")

    with tile.TileContext(nc) as tc, ExitStack() as ctx:
        const = ctx.enter_context(tc.tile_pool(name="const", bufs=1))
        work = ctx.enter_context(tc.tile_pool(name="work", bufs=1))
        ps = ctx.enter_context(tc.tile_pool(name="ps", bufs=1, space="PSUM"))

        xt_sb = const.tile([128, NBLK * 1024], F16)
        wqk_sb = const.tile([128, G * 1024], F16)
        wv_sb = const.tile([128, 4 * 2048], F16)  # quarter-major
        wp_sb = const.tile([128, KC * 1024], F16)
        # DMA order = first-consumption order (block 0's projection stream
        # first).  Issued from the otherwise-idle gpsimd queue: its DMA
        # dispatch is ~25ns vs ~565ns on sync, so the prologue isn't
        # serialized on DMA issue.
        nc.gpsimd.dma_start(xt_sb[:, 0:1024], xt_d[:, 0:1024])
        for gc in range(4):
            nc.gpsimd.dma_start(
                wqk_sb[:, gc * 4096 : (gc + 1) * 4096],
                wqk_d[:, gc * 4096 : (gc + 1) * 4096],
            )
        for qtr in range(4):
            nc.gpsimd.dma_start(
                wv_sb[:, qtr * 2048 : (qtr + 1) * 2048],
                wv_d[:, qtr * 2048 : (qtr + 1) * 2048],
            )
        eye_sb = const.tile([128, 128], F16)
        nc.gpsimd.dma_start(eye_sb, eye_d[:, :])
        for j in range(1, NBLK):
            nc.gpsimd.dma_start(
                xt_sb[:, j * 1024 : (j + 1) * 1024], xt_d[:, j * 1024 : (j + 1) * 1024]
            )
        nc.gpsimd.dma_start(wp_sb, wp_d[:, :])
        bp_sb = const.tile([1, 1024], F32R)
        nc.gpsimd.dma_start(bp_sb, bp_d[:, :])
        ones_sb = const.tile([1, 128], F32R)
        nc.gpsimd.dma_start(ones_sb, ones_d[:, :])

        blk = {}  # j -> (kq, q2, v_sb)

        def alloc_blk(j):
            blk[j] = (
                work.tile([128, 2048], F16, tag="kq", bufs=2, name="kq"),
                work.tile([64, 2048], F16, tag="q2", bufs=2, name="q2"),
                work.tile([128, G * 65], F16, tag="v", bufs=2, name="vsb"),
            )

        # ---- projection micro-items (20 per block) ------------------------
        # ('qk', g): QK for group g -> kq[:, g*128:+128]; ('v', q): quarter.
        def emit_proj_item(j, it):
            kq, q2, v_sb = blk[j]
            kind, idx = it
            if kind == "qk":
                g = idx
                qk_ps = ps.tile([128, 512], F32, tag="mis", name="mis")
                sl = (g % 4) * 128
                for kc in range(KC):
                    nc.tensor.matmul(
                        qk_ps[:, sl : sl + 128],
                        lhsT=wqk_sb[:, g * 1024 + kc * 128 : g * 1024 + (kc + 1) * 128],
                        rhs=xt_sb[:, j * 1024 + kc * 128 : j * 1024 + (kc + 1) * 128],
                        start=(kc == 0),
                        stop=(kc == KC - 1),
                    )
                nc.vector.tensor_copy(
                    kq[:, g * 128 : (g + 1) * 128], qk_ps[:, sl : sl + 128]
                )
                if g == G - 1:
                    nc.sync.dma_start(q2, kq[64:128, :])
            else:
                qtr = idx
                if qtr == 0:
                    nc.vector.memset(v_sb, 1.0)
                v_ps = ps.tile([128, 512], F32, tag="mis", name="mis")
                sl = (qtr % 2) * 256
                for kc in range(KC):
                    nc.tensor.matmul(
                        v_ps[:, sl : sl + 256],
                        lhsT=xt_sb[:, j * 1024 + kc * 128 : j * 1024 + (kc + 1) * 128],
                        rhs=wv_sb[:, qtr * 2048 + kc * 256 : qtr * 2048 + (kc + 1) * 256],
                        start=(kc == 0),
                        stop=(kc == KC - 1),
                    )
                o = v_sb.rearrange("p (a e) -> p a e", e=65)[
                    :, qtr * 4 : (qtr + 1) * 4, 0:64
                ]
                nc.vector.tensor_copy(
                    o, v_ps[:, sl : sl + 256].rearrange("p (a e) -> p a e", e=64)
                )

        def proj_items():
            return [("qk", g) for g in range(G)] + [("v", q) for q in range(4)]

        # ---- per-block attention pieces -----------------------------------
        def emit_scores(j, qh, i):
            kq, q2, _ = blk[j]
            s_t = ps.tile([128, 1024], F32, tag="s", bufs=2, name="st")
            for half in range(2):
                nc.tensor.matmul(
                    s_t[:, half * 512 : (half + 1) * 512],
                    lhsT=kq[0:64, i * 128 : (i + 1) * 128],
                    rhs=q2[:, qh * 1024 + half * 512 : qh * 1024 + half * 512 + 512],
                    start=True,
                    stop=True,
                )
            if i == 0:
                es_t = work.tile([128, 1024], F16, tag="es0", bufs=2, name="es0")
            else:
                es_t = work.tile([128, 1024], F16, tag="es", bufs=3, name="es")
            nc.scalar.activation(es_t, s_t, EXP, scale=0.125)
            return es_t

        def emit_pv(j, qh, i, es_t, ctxE, ctxO):
            v_sb = blk[j][2]
            for tl in range(8):
                ctx_t = ctxE if tl % 2 == 0 else ctxO
                sl = (tl // 2) * 65
                nc.tensor.matmul(
                    ctx_t[:, sl : sl + 65],
                    lhsT=es_t[:, tl * 128 : (tl + 1) * 128],
                    rhs=v_sb[:, i * 65 : i * 65 + 65],
                    start=(i == 0),
                    stop=(i == G - 1 and tl >= 6),
                )

        def emit_readd(j, es0, ctxE, ctxO):
            v_sb = blk[j][2]
            for tl in range(6):
                ctx_t = ctxE if tl % 2 == 0 else ctxO
                sl = (tl // 2) * 65
                nc.tensor.matmul(
                    ctx_t[:, sl : sl + 65],
                    lhsT=es0[:, tl * 128 : (tl + 1) * 128],
                    rhs=v_sb[:, 0:65],
                    start=False,
                    stop=True,
                )

        def emit_norm(ctxE, ctxO):
            ctxn = work.tile([128, 512], F16, tag="ctxn", bufs=2, name="ctxn")
            for tl in range(8):
                ctx_t = ctxE if tl % 2 == 0 else ctxO
                sl = (tl // 2) * 65
                inv = work.tile([128, 1], F32, tag="inv", bufs=4, name="inv")
                nc.vector.reciprocal(inv, ctx_t[:, sl + 64 : sl + 65])
                nc.vector.tensor_scalar_mul(
                    ctxn[:, tl * 64 : (tl + 1) * 64], ctx_t[:, sl : sl + 64], inv
                )
            return ctxn

        def emit_tr(qh, ctxn):
            psT2 = ps.tile([128, 512], F16, tag="pt", name="pt")
            for tl in range(8):
                t = qh * 8 + tl
                dst = (
                    psT2[0:64, (tl // 2) * 128 : (tl // 2) * 128 + 128]
                    if t % 2 == 0
                    else psT2[64:128, (tl // 2) * 128 : (tl // 2) * 128 + 128]
                )
                nc.tensor.transpose(dst, ctxn[:, tl * 64 : (tl + 1) * 64], eye_sb)
            return psT2

        def emit_stack(qh, psT2, ctxT2):
            nc.vector.tensor_copy(ctxT2[:, qh * 512 : (qh + 1) * 512], psT2)

        def emit_outproj_half(hlf, part, ctxT2, psO):
            # part 0: contraction chunks 0..3 (start); part 1: 4..7 + bias
            for i in range(part * 4, part * 4 + 4):
                nc.tensor.matmul(
                    psO,
                    lhsT=ctxT2[:, i * 128 : (i + 1) * 128],
                    rhs=wp_sb[:, i * 1024 + hlf * 512 : i * 1024 + hlf * 512 + 512],
                    start=(i == 0),
                    stop=False,
                )
            if part == 1:
                nc.tensor.matmul(
                    psO,
                    lhsT=ones_sb[:, 0:128],
                    rhs=bp_sb[:, hlf * 512 : hlf * 512 + 512],
                    start=False,
                    stop=True,
                )

        # ---- flat pipeline ------------------------------------------------
        # per window j: 32 score steps; PV lags 2; proj items of block j+1
        # spread one per step; block-(j-1) tail consumed at steps 0/1;
        # block-end chain pre-emits the first two score steps of block j+1.
        STEPS = [(qh, i) for qh in range(2) for i in range(G)]

        alloc_blk(0)
        for it in proj_items():
            emit_proj_item(0, it)

        tail = None  # (j, ctxT2, out_sb) pending second output half
        pre_scored = {}  # (j, qh, i) -> es tile, for steps emitted early

        for j in range(NBLK):
            kq, q2, v_sb = blk[j]
            ctxT2 = work.tile([128, 1024], F16, tag="ctxT2", bufs=2, name="ctxT2")
            out_sb = work.tile([128, 1024], F32, tag="osb", bufs=2, name="osb")
            next_j = j + 1 if j + 1 < NBLK else None
            if next_j is not None:
                alloc_blk(next_j)
            pitems = deque(proj_items() if next_j is not None else [])
            # proj item steps: qh0 i=2..15, qh1 i=2..7
            pslots = {(0, i) for i in range(2, 16)} | {(1, i) for i in range(2, 8)}

            pv_lag = deque()
            ctx_cur = {}
            es0_cur = {}
            ctxn_cur = {}

            for qh, i in STEPS:
                if i == 0:
                    ctx_cur[qh] = (
                        ps.tile([128, 512], F32, tag="ctxE", name="ctxE"),
                        ps.tile([128, 512], F32, tag="ctxO", name="ctxO"),
                    )
                if (j, qh, i) in pre_scored:
                    es_t = pre_scored.pop((j, qh, i))
                else:
                    es_t = emit_scores(j, qh, i)
                if i == 0:
                    es0_cur[qh] = es_t
                pv_lag.append((qh, i, es_t))

                if tail is not None and qh == 0 and i in (0, 1):
                    # previous block's second output half + store
                    tj, tctxT2, tout, tpsO = tail
                    if i == 0:
                        tpsO = ps.tile([128, 512], F32, tag="mis", name="mis")
                        tail = (tj, tctxT2, tout, tpsO)
                    emit_outproj_half(1, i, tctxT2, tpsO)
                    if i == 1:
                        nc.vector.tensor_copy(tout[:, 512:1024], tpsO)
                        nc.sync.dma_start(out_d[tj * 128 : (tj + 1) * 128, :], tout)
                        tail = None
                if len(pv_lag) > 2:
                    pqh, pi, pes = pv_lag.popleft()
                    emit_pv(j, pqh, pi, pes, *ctx_cur[pqh])
                if pitems and (qh, i) in pslots:
                    emit_proj_item(next_j, pitems.popleft())
                if qh == 1 and i == 1:
                    # qh0 fully accumulated (PV(0,15) just emitted above)
                    emit_readd(j, es0_cur[0], *ctx_cur[0])
                    ctxn_cur[0] = emit_norm(*ctx_cur[0])
                if qh == 1 and i == 3:
                    psT2 = emit_tr(0, ctxn_cur[0])
                    emit_stack(0, psT2, ctxT2)

            # ---- block-end chain ----
            while pv_lag:
                pqh, pi, pes = pv_lag.popleft()
                emit_pv(j, pqh, pi, pes, *ctx_cur[pqh])
            emit_readd(j, es0_cur[1], *ctx_cur[1])
            # first output half, chunks 0..3 (qh0 data): fills the norm gap
            psO = ps.tile([128, 512], F32, tag="mis", name="mis")
            emit_outproj_half(0, 0, ctxT2, psO)
            ctxn1 = emit_norm(*ctx_cur[1])
            psT2 = emit_tr(1, ctxn1)
            if next_j is not None:
                pre_scored[(next_j, 0, 0)] = emit_scores(next_j, 0, 0)
            emit_stack(1, psT2, ctxT2)
            emit_outproj_half(0, 1, ctxT2, psO)
            nc.vector.tensor_copy(out_sb[:, 0:512], psO)
            if next_j is not None:
                pre_scored[(next_j, 0, 1)] = emit_scores(next_j, 0, 1)
                tail = (j, ctxT2, out_sb, None)
            else:
                psO = ps.tile([128, 512], F32, tag="mis", name="mis")
                emit_outproj_half(1, 0, ctxT2, psO)
                emit_outproj_half(1, 1, ctxT2, psO)
                nc.vector.tensor_copy(out_sb[:, 512:1024], psO)
                nc.sync.dma_start(out_d[j * 128 : (j + 1) * 128, :], out_sb)
            del blk[j]

    nc.compile()
    return nc


_compiled = {}


def kernel(x, Wq, Wk, Wv, Wp, bp):
    from concourse.bass_utils import run_bass_kernel_spmd

    x = np.asarray(x, dtype=np.float32)
    Wq = np.asarray(Wq, dtype=np.float32)
    Wk = np.asarray(Wk, dtype=np.float32)
    Wv = np.asarray(Wv, dtype=np.float32)
    Wp = np.asarray(Wp, dtype=np.float32)
    bp = np.asarray(bp, dtype=np.float32)

    f16 = np.float16

    # wqk2 g-major: [c, g, dd] -> [128 (c%128... c = kc*128+p), g*1024 + kc*128 + dd]
    wqk = np.empty((IN_DIM, G, 128), np.float32)
    wqk[:, :, :64] = Wk.reshape(IN_DIM, G, 64)
    wqk[:, :, 64:] = Wq.reshape(IN_DIM, G, 64)
    # [kc, p, g, dd] -> p on partitions, cols g*1024 + kc*128 + dd
    wqk_sb = (
        wqk.reshape(KC, 128, G, 128).transpose(1, 2, 0, 3).reshape(128, G * 1024)
    ).astype(f16)
    # quarter-major: [128, qtr*2048 + kc*256 + o]
    wv_sb = (
        Wv.reshape(KC, 128, 4, 256).transpose(1, 2, 0, 3).reshape(128, 4 * 2048)
    ).astype(f16)
    wp_sb = (
        Wp.reshape(KC, 128, 1024).transpose(1, 0, 2).reshape(128, KC * 1024)
    ).astype(f16)
    bp_sb = bp.reshape(1, 1024).astype(np.float32)
    eye = np.eye(128, dtype=f16)

    x_flat = x.reshape(B * S, IN_DIM)
    in_maps = []
    for c in range(N_CORES):
        slab = x_flat[c * 1024 : (c + 1) * 1024]  # [1024 rows, 1024 c]
        xt = np.ascontiguousarray(slab.T)  # [c, row]
        # block-major: [kc, p, j, r] -> [128, j*1024 + kc*128 + r]
        xt_sb = (
            xt.reshape(KC, 128, NBLK, 128).transpose(1, 2, 0, 3).reshape(128, NBLK * 1024)
        ).astype(f16)
        in_maps.append(
            {
                "xt": xt_sb,
                "wqk": wqk_sb,
                "wv": wv_sb,
                "wp": wp_sb,
                "bp": bp_sb,
                "ones": np.ones((1, 128), np.float32),
                "eye": eye,
            }
        )

    if "nc" not in _compiled:
        _compiled["nc"] = _build_nc()
    nc = _compiled["nc"]

    res = run_bass_kernel_spmd(nc, in_maps, list(range(N_CORES)))

    out = np.empty((B * S, OUT_DIM), np.float32)
    for c in range(N_CORES):
        out[c * 1024 : (c + 1) * 1024] = res.results[c]["out"]
    return out.reshape(B, S, OUT_DIM)
